# revision 6
# baseline (speedup 1.0000x reference)
"""Trainium2 Bass kernel for nn_DechunkingLayer.

Full-input contract: kernel(z, p, b, original_len) with
  z [8, 1024, 1024] f32, p [8, 4096] f32, b [8, 4096] i32  ->  [8, 4096, 1024] f32

Sharding: data-parallel over batch — core i processes row i (cumsum / gather /
roll are independent per batch row).

Per-core algorithm (see build_nc):
  idx = clip(cumsum(b) - b, 0, Lz-1)   # scan via tensor_tensor_scan + [32x32]
                                       # strict-triangular matmul for chunk offsets
  up[t] = z[idx[t]]                    # gpsimd indirect-DMA gather, 128 rows/tile
  rolled = partition-shift(up)         # PE superdiagonal matmul + rank-1 halo
  out = p*up + (1-p)*rolled            # ACT per-partition scale + fused DVE
                                       # scalar_tensor_tensor; out[0] = up[0]
"""

import numpy as np

import concourse.bass as bass
import concourse.bacc as bacc
import concourse.tile as tile
from concourse import mybir
from concourse.bass_utils import run_bass_kernel_spmd

P = 128       # partitions / t-tile height
G = 32        # chunks = T // P
T = 4096
LZ = 1024
D = 1024
N_CORES = 8

F32 = mybir.dt.float32
F32R = mybir.dt.float32r
I32 = mybir.dt.int32
ALU = mybir.AluOpType
ACTF = mybir.ActivationFunctionType

# shift-matmul mode:
#   "fp32_fixup": fp32 superdiag matmul for rows 1..127; halo row 0 via a 4KB
#                 SBUF->SBUF DMA of prev tile's last row + a [1,D] fused stt.
#   "f32r_halo":  float32r matmuls (1 cyc/row) incl. rank-1 halo matmul.
SHIFT_MODE = "fp32_fixup"


def _const_inputs() -> dict[str, np.ndarray]:
    return {
        "s_sub": np.eye(P, k=1, dtype=np.float32),            # lhsT[k,m]=1 iff k==m-1
        "oh127": np.eye(P, 1, k=-(P - 1), dtype=np.float32),  # [P,1], 1 at k=127
        "su32": np.triu(np.ones((G, G), dtype=np.float32), 1),
        "id32": np.eye(G, dtype=np.float32),
    }


def build_nc(shift_mode: str | None = None) -> bacc.Bacc:
    if shift_mode is None:
        shift_mode = SHIFT_MODE
    use_f32r = shift_mode == "f32r_halo"
    nc = bacc.Bacc("TRN2", target_bir_lowering=False, debug=False)

    zdt = F32R if use_f32r else F32
    z_d = nc.dram_tensor("z", [LZ, D], zdt, kind="ExternalInput")
    p_d = nc.dram_tensor("p", [G, P], F32, kind="ExternalInput")
    b_d = nc.dram_tensor("b", [G, P], I32, kind="ExternalInput")
    ssub_d = nc.dram_tensor("s_sub", [P, P], F32, kind="ExternalInput")
    oh127_d = nc.dram_tensor("oh127", [P, 1], F32, kind="ExternalInput")
    su32_d = nc.dram_tensor("su32", [G, G], F32, kind="ExternalInput")
    id32_d = nc.dram_tensor("id32", [G, G], F32, kind="ExternalInput")
    out_d = nc.dram_tensor("out", [T, D], F32, kind="ExternalOutput")

    mm_dt = F32R if use_f32r else F32

    with tile.TileContext(nc) as tc:
        with (
            tc.tile_pool(name="consts", bufs=1) as cpool,
            tc.tile_pool(name="small", bufs=1) as spool,
            tc.tile_pool(name="spsum", bufs=1, space="PSUM") as sppool,
            tc.tile_pool(name="up", bufs=4) as upool,
            tc.tile_pool(name="t1", bufs=3) as tpool,
            tc.tile_pool(name="outp", bufs=3) as opool,
            tc.tile_pool(name="lastb", bufs=3) as lpool,
            tc.tile_pool(name="psum", bufs=2, space="PSUM") as ppool,
        ):
            # ---- constants ----
            ssub = cpool.tile([P, P], F32)
            nc.sync.dma_start(ssub[:], ssub_d[:, :])
            oh127 = cpool.tile([P, 1], F32)
            nc.sync.dma_start(oh127[:], oh127_d[:, :])
            su32 = cpool.tile([G, G], F32)
            nc.sync.dma_start(su32[:], su32_d[:, :])
            id32 = cpool.tile([G, G], F32)
            nc.sync.dma_start(id32[:], id32_d[:, :])

            # ---- stage A: idx / p / q in column-major [P, G] ----
            b_nat = spool.tile([G, P], I32)
            nc.sync.dma_start(b_nat[:], b_d[:, :])
            p_nat = spool.tile([G, P], F32)
            nc.sync.dma_start(p_nat[:], p_d[:, :])

            b_f = spool.tile([G, P], F32)
            nc.vector.tensor_copy(b_f[:], b_nat[:])
            zer = spool.tile([G, P], F32)
            nc.vector.memset(zer[:], 0.0)
            ws = spool.tile([G, P], F32)
            nc.vector.tensor_tensor_scan(
                ws[:], zer[:], b_f[:], 0.0, op0=ALU.add, op1=ALU.add
            )
            offs_ps = sppool.tile([G, 1], F32)
            nc.tensor.matmul(
                offs_ps[:], lhsT=su32[:], rhs=ws[:, P - 1 : P], start=True, stop=True
            )
            idx_nat = spool.tile([G, P], F32)
            nc.vector.scalar_tensor_tensor(
                idx_nat[:],
                in0=ws[:],
                scalar=offs_ps[:, 0:1],
                in1=b_f[:],
                op0=ALU.add,
                op1=ALU.subtract,
            )
            nc.vector.tensor_scalar_min(idx_nat[:], idx_nat[:], float(LZ - 1))
            q_nat = spool.tile([G, P], F32)
            nc.vector.tensor_scalar(
                q_nat[:], p_nat[:], -1.0, 1.0, op0=ALU.mult, op1=ALU.add
            )

            # transposes [G, P] -> [P, G]
            idx_ps = sppool.tile([P, G], F32)
            nc.tensor.transpose(idx_ps[:], idx_nat[:], id32[:])
            idx_cm = spool.tile([P, G], I32)
            nc.vector.tensor_copy(idx_cm[:], idx_ps[:])

            p_ps = sppool.tile([P, G], F32)
            nc.tensor.transpose(p_ps[:], p_nat[:], id32[:])
            p_cm = spool.tile([P, G], F32)
            nc.vector.tensor_copy(p_cm[:], p_ps[:])

            q_ps = sppool.tile([P, G], F32)
            nc.tensor.transpose(q_ps[:], q_nat[:], id32[:])
            q_cm = spool.tile([P, G], F32)
            nc.vector.tensor_copy(q_cm[:], q_ps[:])

            # out[0] must equal up[0]: force p=1 there (rolled contribution is 0)
            nc.vector.memset(p_cm[0:1, 0:1], 1.0)

            # ---- stage B: per t-tile gather + shift + blend ----
            prev_up = None
            last_buf = None
            for g in range(G):
                up = upool.tile([P, D], zdt)
                nc.gpsimd.indirect_dma_start(
                    out=up[:],
                    out_offset=None,
                    in_=z_d[:, :],
                    in_offset=bass.IndirectOffsetOnAxis(ap=idx_cm[:, g : g + 1], axis=0),
                )

                ps = ppool.tile([P, D], F32)
                for h in range(0, D, 512):
                    nc.tensor.matmul(
                        ps[:, h : h + 512],
                        lhsT=ssub[:].bitcast(mm_dt),
                        rhs=up[:, h : h + 512].bitcast(mm_dt),
                        start=True,
                        stop=True,
                    )
                if use_f32r and g > 0:
                    # overwrite row 0 (shift matmul left it at 0) with the halo row
                    for h in range(0, D, 512):
                        nc.tensor.matmul(
                            ps[0:1, h : h + 512],
                            lhsT=oh127[:].bitcast(mm_dt),
                            rhs=prev_up[:, h : h + 512].bitcast(mm_dt),
                            start=True,
                            stop=True,
                        )

                t1 = tpool.tile([P, D], F32)
                nc.scalar.activation(
                    t1[:], up[:].bitcast(F32), func=ACTF.Copy, scale=p_cm[:, g : g + 1]
                )
                ot = opool.tile([P, D], F32)
                nc.vector.scalar_tensor_tensor(
                    ot[:],
                    in0=ps[:],
                    scalar=q_cm[:, g : g + 1],
                    in1=t1[:],
                    op0=ALU.mult,
                    op1=ALU.add,
                )
                if not use_f32r and g > 0:
                    # row-0 halo: out[t0] = q[t0]*up[t0-1] + p[t0]*up[t0]
                    nc.vector.scalar_tensor_tensor(
                        ot[0:1, :],
                        in0=last_buf[:],
                        scalar=q_cm[0:1, g : g + 1],
                        in1=t1[0:1, :],
                        op0=ALU.mult,
                        op1=ALU.add,
                    )
                nc.sync.dma_start(out_d[g * P : (g + 1) * P, :], ot[:])
                if not use_f32r and g < G - 1:
                    last_buf = lpool.tile([1, D], F32)
                    nc.sync.dma_start(last_buf[:], up[P - 1 : P, :].bitcast(F32))
                prev_up = up

    nc.compile()
    return nc


_NC_CACHE: dict[str, bacc.Bacc] = {}


def get_nc(shift_mode: str | None = None) -> bacc.Bacc:
    if shift_mode is None:
        shift_mode = SHIFT_MODE
    if shift_mode not in _NC_CACHE:
        _NC_CACHE[shift_mode] = build_nc(shift_mode)
    return _NC_CACHE[shift_mode]


def make_in_maps(z: np.ndarray, p: np.ndarray, b: np.ndarray) -> list[dict]:
    consts = _const_inputs()
    maps = []
    for i in range(N_CORES):
        m = {
            "z": np.ascontiguousarray(z[i], dtype=np.float32),
            "p": np.ascontiguousarray(p[i].reshape(G, P), dtype=np.float32),
            "b": np.ascontiguousarray(b[i].reshape(G, P), dtype=np.int32),
        }
        m.update(consts)
        maps.append(m)
    return maps


def run(z, p, b, **spmd_kwargs):
    nc = get_nc()
    in_maps = make_in_maps(z, p, b)
    res = run_bass_kernel_spmd(nc, in_maps, core_ids=list(range(N_CORES)), **spmd_kwargs)
    out = np.stack([res.results[i]["out"] for i in range(N_CORES)], axis=0)
    return out, res


def kernel(z, p, b, original_len=None, **_ignored) -> np.ndarray:
    z = np.asarray(z)
    p = np.asarray(p)
    b = np.asarray(b)
    assert z.shape == (N_CORES, LZ, D), z.shape
    assert p.shape == (N_CORES, T), p.shape
    assert b.shape == (N_CORES, T), b.shape
    out, _ = run(z, p, b)
    return out.astype(np.float32, copy=False)
